# revision 1
# baseline (speedup 1.0000x reference)
"""Trainium2 Bass kernel for nn_JointSelfAttentionLayer.

Math restructuring (both outputs are sequence-means):
  C[b]    = (1/SC) * colsum_b @ x_d[b] @ W_vd,  colsum_b[t] = sum_s softmax(logits)[s,t]/sqrt(D)
  Dout[b] = (1/(SD*sqrt(D))) * (sum_s x_c[b,s,:]) @ W_vc      (softmax rows sum to 1)
so the only heavy work is logits = x_c @ (W_qc @ W_kd^T) @ x_d^T plus a
streaming softmax column-sum. Never materializes Q, K, V_c, V_d, or A@V.

f16 hi/lo 3-pass matmuls give fp32-grade products (probed 3e-5 abs on
K=1024 N(0,1) dots) at 3x the fp32 matmul rate.
"""
import numpy as np
from contextlib import ExitStack

B, SC, SD, D = 8, 2048, 2048, 1024
P = 128
DB = D // P            # 8 d-blocks
TB = SD // P           # 16 t-blocks
SBK = SC // P          # 16 s-blocks
CH = 512
NCH = SC // CH         # 4 chunks
INV_SQRT_D = 1.0 / 32.0


def _split_excess_waits(nc, mybir, max_waits=1):
    n = 0
    ctr = [0]
    for fn in nc.m.functions:
        for bb in fn.blocks:
            out = []
            changed = False
            for inst in bb.instructions:
                si = inst.sync_info
                ws = list(si.on_wait) if (si and si.on_wait) else []
                if len(ws) > max_waits and inst.engine != mybir.EngineType.Unassigned:
                    keep = ws[:max_waits]
                    excess = ws[max_waits:]
                    for i in range(0, len(excess), max_waits):
                        chunk = excess[i:i + max_waits]
                        nop = mybir.InstNoOp(name=f"ws_{ctr[0]}", ins=[], outs=[])
                        ctr[0] += 1
                        nop.engine = inst.engine
                        nop.sync_info = mybir.SyncInfo(on_wait=chunk, on_update=[])
                        out.append(nop)
                    inst.sync_info = mybir.SyncInfo(
                        on_wait=keep, on_update=list(si.on_update or []))
                    changed = True
                    n += 1
                out.append(inst)
            if changed:
                bb.instructions = out
    return n


def _build(repeats=1):
    import concourse.bass as bass
    import concourse.tile as tile
    from concourse import mybir
    from concourse.masks import make_identity

    F32 = mybir.dt.float32
    F16 = mybir.dt.float16
    Act = mybir.ActivationFunctionType
    Alu = mybir.AluOpType
    AxX = mybir.AxisListType.X

    nc = bass.Bass("TRN2", target_bir_lowering=False, debug=False, num_devices=8)
    xc = nc.dram_tensor("x_c", [SC, D], F32, kind="ExternalInput").ap()
    xd = nc.dram_tensor("x_d", [SD, D], F32, kind="ExternalInput").ap()
    wqc = nc.dram_tensor("W_qc", [D, D], F32, kind="ExternalInput").ap()
    wvc = nc.dram_tensor("W_vc", [D, D], F32, kind="ExternalInput").ap()
    wkd = nc.dram_tensor("W_kd", [D, D], F32, kind="ExternalInput").ap()
    wvd = nc.dram_tensor("W_vd", [D, D], F32, kind="ExternalInput").ap()
    out_d = nc.dram_tensor("out", [P, 16], F32, kind="ExternalOutput").ap()

    with tile.TileContext(nc) as tc, ExitStack() as ctx:
        const = ctx.enter_context(tc.tile_pool(name="const", bufs=1))
        ident = const.tile([P, P], F32, name="ident")
        make_identity(nc, ident[:])
        cp = const.tile([P, SD], F32, name="cp")
        xsum = const.tile([P, DB], F32, name="xsum")
        out_sb = const.tile([P, 16], F32, name="out_sb")
        colsT = const.tile([P, TB], F32, name="colsT")

        for _r in range(repeats):
            nc.gpsimd.memset(cp[:], 0.0)
            nc.gpsimd.memset(xsum[:], 0.0)
            with tc.tile_pool(name=f"gtp_{_r}", bufs=1) as gtp:
                gt_h = [gtp.tile([P, SC], F16, name=f"gt_h{j}_{_r}") for j in range(DB)]
                gt_l = [gtp.tile([P, SC], F16, name=f"gt_l{j}_{_r}") for j in range(DB)]

                # ---- phase 1 + 2 under wqk scope ----
                with tc.tile_pool(name=f"wqkp_{_r}", bufs=1) as wqkp:
                    wqk_h = [wqkp.tile([P, D], F16, name=f"wqk_h{i}_{_r}") for i in range(DB)]
                    wqk_l = [wqkp.tile([P, D], F16, name=f"wqk_l{i}_{_r}") for i in range(DB)]

                    # phase 1: Wqk = W_qc @ W_kd^T in fp32
                    with tc.tile_pool(name=f"ph1_{_r}", bufs=1) as ph1, \
                         tc.tile_pool(name=f"ph1w_{_r}", bufs=2) as ph1w:
                        wqcT = [ph1.tile([P, D], F32, name=f"wqcT{k}_{_r}") for k in range(DB)]
                        wkdT = [ph1.tile([P, D], F32, name=f"wkdT{k}_{_r}") for k in range(DB)]
                        with tc.tile_pool(name=f"ph1ps_{_r}", bufs=4, space="PSUM") as ph1ps:
                            for (dram, dstT, nm) in ((wqc, wqcT, "q"), (wkd, wkdT, "k")):
                                for ih in range(2):
                                    wts = []
                                    for i4 in range(4):
                                        i = ih * 4 + i4
                                        wt = ph1w.tile([P, D], F32, name=f"w{nm}_{i}_{_r}", tag=f"w{i4}")
                                        nc.gpsimd.dma_start(wt[:], dram[i * P:(i + 1) * P, :])
                                        wts.append(wt)
                                    for k in range(DB):
                                        tp = ph1ps.tile([P, 4 * P], F32, name=f"tp{nm}_{ih}_{k}_{_r}", tag="tp")
                                        for i4 in range(4):
                                            nc.tensor.transpose(tp[:, i4 * P:(i4 + 1) * P],
                                                                wts[i4][:, k * P:(k + 1) * P], ident[:])
                                        nc.scalar.activation(
                                            dstT[k][:, ih * 4 * P:(ih + 1) * 4 * P], tp[:], Act.Copy)
                        tmp1 = ph1.tile([P, CH], F32, name=f"tmp1_{_r}")
                        with tc.tile_pool(name=f"ph1ps2_{_r}", bufs=2, space="PSUM") as ph1ps2:
                            for i in range(DB):
                                for c in range(2):
                                    pq = ph1ps2.tile([P, CH], F32, name=f"pq_{i}_{c}_{_r}", tag="pq")
                                    for k in range(DB):
                                        nc.tensor.matmul(
                                            pq[:], wqcT[k][:, i * P:(i + 1) * P],
                                            wkdT[k][:, c * CH:(c + 1) * CH],
                                            start=(k == 0), stop=(k == DB - 1))
                                    sl = slice(c * CH, (c + 1) * CH)
                                    nc.scalar.activation(wqk_h[i][:, sl], pq[:], Act.Copy)
                                    nc.vector.tensor_copy(tmp1[:], wqk_h[i][:, sl])
                                    nc.vector.tensor_sub(wqk_l[i][:, sl], pq[:], tmp1[:])

                    # phase 2: stream x_c -> GT (f16x3) + xsum
                    with tc.tile_pool(name=f"ph2_{_r}", bufs=2) as ph2, \
                         tc.tile_pool(name=f"ph2ps_{_r}", bufs=6, space="PSUM") as ps_t, \
                         tc.tile_pool(name=f"ph2ps2_{_r}", bufs=2, space="PSUM") as ps_g:
                        for c in range(NCH):
                            xh = [ph2.tile([P, CH], F16, name=f"xh{c}_{j}_{_r}", tag=f"xh{j}")
                                  for j in range(DB)]
                            xl = [ph2.tile([P, CH], F16, name=f"xl{c}_{j}_{_r}", tag=f"xl{j}")
                                  for j in range(DB)]
                            tmp = ph2.tile([P, CH], F32, name=f"tmp{c}_{_r}", tag="tmp")
                            red = ph2.tile([P, 1], F32, name=f"red{c}_{_r}", tag="red")
                            xts = []
                            for s in range(4):
                                xt = ph2.tile([P, D], F32, name=f"xt{c}_{s}_{_r}", tag=f"xt{s}")
                                nc.gpsimd.dma_start(xt[:], xc[c * CH + s * P:c * CH + (s + 1) * P, :])
                                xts.append(xt)
                            for j in range(DB):
                                tp = ps_t.tile([P, CH], F32, name=f"t2_{c}_{j}_{_r}", tag="tp")
                                for s in range(4):
                                    nc.tensor.transpose(tp[:, s * P:(s + 1) * P],
                                                        xts[s][:, j * P:(j + 1) * P], ident[:])
                                nc.scalar.activation(xh[j][:], tp[:], Act.Copy)
                                nc.vector.tensor_copy(tmp[:], xh[j][:])
                                nc.vector.tensor_sub(xl[j][:], tp[:], tmp[:])
                                nc.vector.tensor_reduce(red[:], tp[:], AxX, Alu.add)
                                nc.vector.tensor_add(xsum[:, j:j + 1], xsum[:, j:j + 1], red[:])
                            tmpg = ph2.tile([P, CH], F32, name=f"tmpg{c}_{_r}", tag="tmpg")
                            for jp in range(DB):
                                pg = ps_g.tile([P, CH], F32, name=f"pg{c}_{jp}_{_r}", tag="pg")
                                first = True
                                for (wt_, xt_) in ((wqk_h, xh), (wqk_h, xl), (wqk_l, xh)):
                                    for i in range(DB):
                                        nc.tensor.matmul(
                                            pg[:], wt_[i][:, jp * P:(jp + 1) * P], xt_[i][:],
                                            start=first, stop=False)
                                        first = False
                                sl = slice(c * CH, (c + 1) * CH)
                                nc.scalar.activation(gt_h[jp][:, sl], pg[:], Act.Copy)
                                nc.vector.tensor_copy(tmpg[:], gt_h[jp][:, sl])
                                nc.vector.tensor_sub(gt_l[jp][:, sl], pg[:], tmpg[:])

                # ---- phase 3: stream x_d -> x_dT hi/lo ----
                with tc.tile_pool(name=f"xdtp_{_r}", bufs=1) as xdtp:
                    xdt_h = [xdtp.tile([P, SD], F16, name=f"xdt_h{j}_{_r}") for j in range(DB)]
                    xdt_l = [xdtp.tile([P, SD], F16, name=f"xdt_l{j}_{_r}") for j in range(DB)]
                    with tc.tile_pool(name=f"ph3_{_r}", bufs=2) as ph3, \
                         tc.tile_pool(name=f"ph3ps_{_r}", bufs=6, space="PSUM") as ps3:
                        for c in range(NCH):
                            tmp = ph3.tile([P, CH], F32, name=f"t3m{c}_{_r}", tag="tmp")
                            xts = []
                            for s in range(4):
                                xt = ph3.tile([P, D], F32, name=f"x3t{c}_{s}_{_r}", tag=f"xt{s}")
                                nc.gpsimd.dma_start(xt[:], xd[c * CH + s * P:c * CH + (s + 1) * P, :])
                                xts.append(xt)
                            for j in range(DB):
                                tp = ps3.tile([P, CH], F32, name=f"t3_{c}_{j}_{_r}", tag="tp")
                                for s in range(4):
                                    nc.tensor.transpose(tp[:, s * P:(s + 1) * P],
                                                        xts[s][:, j * P:(j + 1) * P], ident[:])
                                csl = slice(c * CH, (c + 1) * CH)
                                nc.scalar.activation(xdt_h[j][:, csl], tp[:], Act.Copy)
                                nc.vector.tensor_copy(tmp[:], xdt_h[j][:, csl])
                                nc.vector.tensor_sub(xdt_l[j][:, csl], tp[:], tmp[:])

                    # ---- phase 4: logits + softmax colsum ----
                    with tc.tile_pool(name=f"ph4_{_r}", bufs=2) as ph4, \
                         tc.tile_pool(name=f"ph4s_{_r}", bufs=2) as ph4s, \
                         tc.tile_pool(name=f"ph4ps_{_r}", bufs=2, space="PSUM") as ph4ps:
                        for sb in range(SBK):
                            L = ph4ps.tile([P, SD], F32, name=f"L{sb}_{_r}", tag="L")
                            ssl = slice(sb * P, (sb + 1) * P)
                            for c in range(NCH):
                                tsl = slice(c * CH, (c + 1) * CH)
                                first = True
                                for (gt, xdt) in ((gt_h, xdt_h), (gt_h, xdt_l), (gt_l, xdt_h)):
                                    for j in range(DB):
                                        nc.tensor.matmul(
                                            L[:, tsl], gt[j][:, ssl], xdt[j][:, tsl],
                                            start=first, stop=False)
                                        first = False
                            mx = ph4s.tile([P, 1], F32, name=f"mx{sb}_{_r}", tag="mx")
                            nc.vector.tensor_reduce(mx[:], L[:], AxX, Alu.max)
                            negmx = ph4s.tile([P, 1], F32, name=f"negmx{sb}_{_r}", tag="negmx")
                            nc.vector.tensor_scalar_mul(negmx[:], mx[:], -1.0)
                            E = ph4.tile([P, SD], F32, name=f"E{sb}_{_r}", tag="E")
                            rs = ph4s.tile([P, 1], F32, name=f"rs{sb}_{_r}", tag="rs")
                            nc.scalar.activation(E[:], L[:], Act.Exp,
                                                 bias=negmx[:], scale=1.0, accum_out=rs[:])
                            w = ph4s.tile([P, 1], F32, name=f"w{sb}_{_r}", tag="w")
                            nc.vector.reciprocal(w[:], rs[:])
                            Et = ph4.tile([P, SD], F32, name=f"Et{sb}_{_r}", tag="Et")
                            nc.vector.tensor_scalar(Et[:], E[:], w[:], INV_SQRT_D,
                                                    Alu.mult, Alu.mult)
                            nc.vector.tensor_add(cp[:], cp[:], Et[:])

            # ---- phase 5/6: epilogues (gt/xdt pools freed) ----
            with tc.tile_pool(name=f"ph5_{_r}", bufs=2) as ph5, \
                 tc.tile_pool(name=f"ph5c_{_r}", bufs=1) as ph5c, \
                 tc.tile_pool(name=f"ph5ps_{_r}", bufs=2, space="PSUM") as ph5ps, \
                 tc.tile_pool(name=f"ph5ps2_{_r}", bufs=1, space="PSUM") as ph5ps2:
                for t in range(TB):
                    tp = ph5ps.tile([P, P], F32, name=f"cpt{t}_{_r}", tag="cpt")
                    nc.tensor.transpose(tp[:], cp[:, t * P:(t + 1) * P], ident[:])
                    nc.vector.tensor_reduce(colsT[:, t:t + 1], tp[:], AxX, Alu.add)
                pu = ph5ps2.tile([P, DB], F32, name=f"pu_{_r}")
                for t in range(TB):
                    xdn = ph5.tile([P, D], F32, name=f"xdn{t}_{_r}", tag="xdn")
                    nc.gpsimd.dma_start(xdn[:], xd[t * P:(t + 1) * P, :])
                    for j in range(DB):
                        nc.tensor.matmul(pu[:, j:j + 1], xdn[:, j * P:(j + 1) * P],
                                         colsT[:, t:t + 1],
                                         start=(j == 0 and t == 0), stop=False)
                u_sb = ph5c.tile([P, DB], F32, name=f"u_sb_{_r}")
                nc.vector.tensor_copy(u_sb[:], pu[:])
                pc = ph5ps2.tile([P, DB], F32, name=f"pc_{_r}")
                for i in range(DB):
                    wvdt = ph5.tile([P, D], F32, name=f"wvdt{i}_{_r}", tag="wvdt")
                    nc.gpsimd.dma_start(wvdt[:], wvd[i * P:(i + 1) * P, :])
                    for e in range(DB):
                        nc.tensor.matmul(pc[:, e:e + 1], wvdt[:, e * P:(e + 1) * P],
                                         u_sb[:, i:i + 1],
                                         start=(e == 0 and i == 0), stop=False)
                nc.scalar.activation(out_sb[:, 0:DB], pc[:], Act.Copy, scale=1.0 / SC)

                pd = ph5ps2.tile([P, DB], F32, name=f"pd_{_r}")
                for i in range(DB):
                    wvct = ph5.tile([P, D], F32, name=f"wvct{i}_{_r}", tag="wvct")
                    nc.gpsimd.dma_start(wvct[:], wvc[i * P:(i + 1) * P, :])
                    for e in range(DB):
                        nc.tensor.matmul(pd[:, e:e + 1], wvct[:, e * P:(e + 1) * P],
                                         xsum[:, i:i + 1],
                                         start=(e == 0 and i == 0), stop=False)
                nc.scalar.activation(out_sb[:, DB:16], pd[:], Act.Copy,
                                     scale=1.0 / (SD * 32.0))
                nc.scalar.dma_start(out_d[:], out_sb[:])

    _split_excess_waits(nc, mybir)
    return nc


def kernel(x_c, x_d, W_qc, W_vc, W_kd, W_vd):
    from concourse.bass_utils import run_bass_kernel_spmd
    nc = _build()
    in_maps = []
    for b in range(B):
        in_maps.append({
            "x_c": np.ascontiguousarray(x_c[b]),
            "x_d": np.ascontiguousarray(x_d[b]),
            "W_qc": np.asarray(W_qc), "W_vc": np.asarray(W_vc),
            "W_kd": np.asarray(W_kd), "W_vd": np.asarray(W_vd),
        })
    res = run_bass_kernel_spmd(nc, in_maps, list(range(B))).results
    C = np.empty((B, D), dtype=np.float32)
    Dout = np.empty((B, D), dtype=np.float32)
    for b in range(B):
        o = res[b]["out"]
        C[b] = o[:, :DB].T.ravel()
        Dout[b] = o[:, DB:16].T.ravel()
    return (C, Dout)



# revision 3
# speedup vs baseline: 3.0908x; 3.0908x over previous
"""Trainium2 Bass kernel for nn_JointSelfAttentionLayer.

Math restructuring (both outputs are sequence-means):
  C[b]    = (1/SC) * (colsum_b @ x_d[b]) @ W_vd,  colsum_b[t] = sum_s softmax(logits)[s,t]/sqrt(D)
  Dout[b] = (1/(SD*sqrt(D))) * (sum_s x_c[b,s,:]) @ W_vc   (softmax rows sum to 1)
so the only heavy device work is logits = x_c @ G @ x_d^T (G = W_qc @ W_kd^T)
plus a streaming softmax column-sum.

Precision plan (tolerance 2e-2; measured in numpy emulation 1.3e-3):
  - host computes G in fp32, ships G/x_c/x_d as f16 (10MB/core vs 32MB fp32)
  - device does single-pass f16 matmuls (1 cycle/row on the PE)
  - the tiny rank-1 epilogue products (u @ W_vd, xsum @ W_vc) run on host in fp32
x_c^T / x_d^T are produced by hardware DMA transpose (2-byte dtype xbar path),
keeping the PE free for the two big matmuls.
"""
import numpy as np
from contextlib import ExitStack

B, SC, SD, D = 8, 2048, 2048, 1024
P = 128
DB = D // P            # 8 d-blocks
TB = SD // P           # 16 t-blocks
SBK = SC // P          # 16 s-blocks
CH = 512
NCH = SD // CH         # 4 chunks
INV_SQRT_D = 1.0 / 32.0


def _split_excess_waits(nc, mybir, max_waits=1):
    n = 0
    ctr = [0]
    for fn in nc.m.functions:
        for bb in fn.blocks:
            out = []
            changed = False
            for inst in bb.instructions:
                si = inst.sync_info
                ws = list(si.on_wait) if (si and si.on_wait) else []
                if len(ws) > max_waits and inst.engine != mybir.EngineType.Unassigned:
                    keep = ws[:max_waits]
                    excess = ws[max_waits:]
                    for i in range(0, len(excess), max_waits):
                        chunk = excess[i:i + max_waits]
                        nop = mybir.InstNoOp(name=f"ws_{ctr[0]}", ins=[], outs=[])
                        ctr[0] += 1
                        nop.engine = inst.engine
                        nop.sync_info = mybir.SyncInfo(on_wait=chunk, on_update=[])
                        out.append(nop)
                    inst.sync_info = mybir.SyncInfo(
                        on_wait=keep, on_update=list(si.on_update or []))
                    changed = True
                    n += 1
                out.append(inst)
            if changed:
                bb.instructions = out
    return n


def _build():
    import concourse.bass as bass
    import concourse.tile as tile
    from concourse import mybir
    from concourse.masks import make_identity

    F32 = mybir.dt.float32
    F16 = mybir.dt.float16
    Act = mybir.ActivationFunctionType
    Alu = mybir.AluOpType
    AxX = mybir.AxisListType.X

    nc = bass.Bass("TRN2", target_bir_lowering=False, debug=False, num_devices=8)
    xc = nc.dram_tensor("xc", [SC, D], F16, kind="ExternalInput").ap()
    xd = nc.dram_tensor("xd", [SD, D], F16, kind="ExternalInput").ap()
    g = nc.dram_tensor("g", [D, D], F16, kind="ExternalInput").ap()
    out_d = nc.dram_tensor("out", [P, 16], F32, kind="ExternalOutput").ap()

    with tile.TileContext(nc) as tc, ExitStack() as ctx:
        const = ctx.enter_context(tc.tile_pool(name="const", bufs=1))
        ident = const.tile([P, P], F32, name="ident")
        make_identity(nc, ident[:])
        cp = const.tile([P, SD], F32, name="cp")
        nc.gpsimd.memset(cp[:], 0.0)
        xsum = const.tile([P, DB], F32, name="xsum")
        out_sb = const.tile([P, 16], F32, name="out_sb")
        colsT = const.tile([P, TB], F32, name="colsT")
        colsT16 = const.tile([P, TB], F16, name="colsT16")

        big = ctx.enter_context(tc.tile_pool(name="big", bufs=1))
        gw = [big.tile([P, D], F16, name=f"g{i}") for i in range(DB)]
        xcT = [big.tile([P, SC], F16, name=f"xcT{j}") for j in range(DB)]
        xdT = [big.tile([P, SD], F16, name=f"xdT{j}") for j in range(DB)]
        ht = [big.tile([P, SC], F16, name=f"ht{j}") for j in range(DB)]

        # ---- loads: G (copy) + x_c^T / x_d^T (hw dma transpose) ----
        for i in range(DB):
            nc.gpsimd.dma_start(gw[i][:], g[i * P:(i + 1) * P, :])
        # NB: all xbar transposes must stay on ONE queue — concurrent
        # transpose-mode DMAs from two engines corrupt each other.
        for j in range(DB):
            nc.sync.dma_start(xcT[j][:], xc[:, j * P:(j + 1) * P], transpose=True)
        for j in range(DB):
            nc.sync.dma_start(xdT[j][:], xd[:, j * P:(j + 1) * P], transpose=True)

        # xsum[:, j] = sum_s x_c[s, j*128+p]
        for j in range(DB):
            nc.vector.tensor_reduce(xsum[:, j:j + 1], xcT[j][:], AxX, Alu.add)

        # ---- phase 2: HT[jp] = (x_c @ G)^T, f16 single pass ----
        with tc.tile_pool(name="p2ps", bufs=4, space="PSUM") as p2ps:
            for jp in range(DB):
                for c in range(SC // CH):
                    pg = p2ps.tile([P, CH], F32, name=f"pg{jp}_{c}", tag="pg")
                    ssl = slice(c * CH, (c + 1) * CH)
                    for i in range(DB):
                        nc.tensor.matmul(pg[:], gw[i][:, jp * P:(jp + 1) * P],
                                         xcT[i][:, ssl],
                                         start=(i == 0), stop=(i == DB - 1))
                    nc.scalar.activation(ht[jp][:, ssl], pg[:], Act.Copy)

        # ---- phase 4: logits + softmax colsum ----
        with tc.tile_pool(name="p4", bufs=2) as p4, \
             tc.tile_pool(name="p4s", bufs=2) as p4s, \
             tc.tile_pool(name="p4ps", bufs=2, space="PSUM") as p4ps:
            for sb in range(SBK):
                L = p4ps.tile([P, SD], F32, name=f"L{sb}", tag="L")
                ssl = slice(sb * P, (sb + 1) * P)
                for c in range(NCH):
                    tsl = slice(c * CH, (c + 1) * CH)
                    for j in range(DB):
                        nc.tensor.matmul(L[:, tsl], ht[j][:, ssl], xdT[j][:, tsl],
                                         start=(j == 0), stop=(j == DB - 1))
                mx = p4s.tile([P, 1], F32, name=f"mx{sb}", tag="mx")
                nc.vector.tensor_reduce(mx[:], L[:], AxX, Alu.max)
                negmx = p4s.tile([P, 1], F32, name=f"negmx{sb}", tag="negmx")
                nc.vector.tensor_scalar_mul(negmx[:], mx[:], -1.0)
                E = p4.tile([P, SD], F32, name=f"E{sb}", tag="E")
                rs = p4s.tile([P, 1], F32, name=f"rs{sb}", tag="rs")
                nc.scalar.activation(E[:], L[:], Act.Exp,
                                     bias=negmx[:], scale=1.0, accum_out=rs[:])
                w = p4s.tile([P, 1], F32, name=f"w{sb}", tag="w")
                nc.vector.reciprocal(w[:], rs[:])
                Et = p4.tile([P, SD], F32, name=f"Et{sb}", tag="Et")
                nc.vector.tensor_scalar(Et[:], E[:], w[:], INV_SQRT_D,
                                        Alu.mult, Alu.mult)
                nc.vector.tensor_add(cp[:], cp[:], Et[:])

        # ---- phase 5: u = colsum @ x_d (device), weight products on host ----
        with tc.tile_pool(name="p5", bufs=2) as p5, \
             tc.tile_pool(name="p5ps", bufs=2, space="PSUM") as p5ps, \
             tc.tile_pool(name="p5ps2", bufs=1, space="PSUM") as p5ps2:
            for t in range(TB):
                tp = p5ps.tile([P, P], F32, name=f"cpt{t}", tag="cpt")
                nc.tensor.transpose(tp[:], cp[:, t * P:(t + 1) * P], ident[:])
                nc.vector.tensor_reduce(colsT[:, t:t + 1], tp[:], AxX, Alu.add)
            nc.vector.tensor_copy(colsT16[:], colsT[:])
            pu = p5ps2.tile([P, DB], F32, name="pu")
            for t in range(TB):
                xdn = p5.tile([P, D], F16, name=f"xdn{t}", tag="xdn")
                nc.gpsimd.dma_start(xdn[:], xd[t * P:(t + 1) * P, :])
                for j in range(DB):
                    nc.tensor.matmul(pu[:, j:j + 1], xdn[:, j * P:(j + 1) * P],
                                     colsT16[:, t:t + 1],
                                     start=(j == 0 and t == 0), stop=False)
            nc.vector.tensor_copy(out_sb[:, 0:DB], pu[:])
            nc.vector.tensor_copy(out_sb[:, DB:16], xsum[:])
            nc.scalar.dma_start(out_d[:], out_sb[:])

    _split_excess_waits(nc, mybir)
    return nc


def kernel(x_c, x_d, W_qc, W_vc, W_kd, W_vd):
    from concourse.bass_utils import run_bass_kernel_spmd
    f16 = np.float16
    W_qc = np.asarray(W_qc, dtype=np.float32)
    W_vc = np.asarray(W_vc, dtype=np.float32)
    W_kd = np.asarray(W_kd, dtype=np.float32)
    W_vd = np.asarray(W_vd, dtype=np.float32)
    g16 = (W_qc @ W_kd.T).astype(f16)
    xc16 = np.asarray(x_c).astype(f16)
    xd16 = np.asarray(x_d).astype(f16)

    nc = _build()
    in_maps = [{"xc": xc16[b], "xd": xd16[b], "g": g16} for b in range(B)]
    res = run_bass_kernel_spmd(nc, in_maps, list(range(B))).results

    u = np.empty((B, D), dtype=np.float32)
    xs = np.empty((B, D), dtype=np.float32)
    for b in range(B):
        o = res[b]["out"]
        u[b] = o[:, :DB].T.ravel()
        xs[b] = o[:, DB:16].T.ravel()
    C = (u @ W_vd) / SC
    Dout = (xs @ W_vc) / (SD * 32.0)
    return (C, Dout)
